# revision 9
# baseline (speedup 1.0000x reference)
"""DiT block kernel for 8 Trainium2 NeuronCores.

Sharding: core = 4*b + s  (b = batch 0..1, s = token-slice 0..3 of 1024 tokens).
Each core computes the full DiT block for its 1024 tokens; K/V for the whole
batch are recomputed per core (sequence-parallel, no collectives).

Device layout is channel-major ([C, n] "transposed") throughout: all weights
are natural lhsT operands, per-channel modulation vectors are per-partition
scalars, and the host pre-transposes x / post-transposes the output.

Softmax: S^T[k, q] tiles on PSUM, E = exp(scale*S) on ScalarE (3 banks per op),
mask handled by zeroing masked V rows; a per-head mask column appended to V
yields the softmax denominator Z as row 64 of the E@[V|m] matmul.
"""

import os
import numpy as np
import ml_dtypes

try:
    import concourse.bass as bass
except ImportError:  # pragma: no cover
    import sys

    for _p in ("/opt/trn_rl_repo", "/opt/pypackages"):
        if _p not in sys.path:
            sys.path.append(_p)
    import concourse.bass as bass

import concourse.tile as tile
import concourse.mybir as mybir
from concourse import bacc, bass_utils

F32 = mybir.dt.float32
BF16 = mybir.dt.bfloat16
AF = mybir.ActivationFunctionType
ALU = mybir.AluOpType
BF = ml_dtypes.bfloat16

B, N, C = 2, 4096, 512
H, D = 8, 64
P = 128
TOK = 1024            # tokens owned per core
NT = N // 512         # 8 n-tiles over the full batch
NT2 = TOK // 512      # 2 own n-tiles
CO = C // P           # 4 channel chunks
KT_N = N // P         # 32 key chunks
SCALE = float(D) ** -0.5
EPS = 1e-6
EG = 3                # S banks per exp op

LAST_EXEC_NS = None
_CACHE = {}


def _build(loop_n=1):
    nc = bacc.Bacc(
        "TRN2",
        target_bir_lowering=False,
        debug=False,
        enable_asserts=True,
        num_devices=8,
    )

    def din(name, shape, dtype):
        return nc.dram_tensor(name, shape, dtype, kind="ExternalInput").ap()

    xTb = din("xTb", [C, N], BF16)          # bf16 x^T, full batch
    xTo = din("xTo", [C, TOK], F32)         # fp32 x^T, own tokens
    xTob = din("xTob", [C, TOK], BF16)      # bf16 x^T, own tokens
    kqw = din("kqw", [C, 2 * C], BF16)      # qkv_w cols 0:512 (Q) + 512:1024 (K)
    vwa = din("vwa", [C, 260], BF16)        # V weights heads 0-3, 65-interleaved
    vwb = din("vwb", [C, 260], BF16)        # V weights heads 4-7
    vbra = din("vbra", [2, 260], BF16)      # [vbias row; indicator row] heads 0-3
    vbrb = din("vbrb", [2, 260], BF16)
    mrow = din("mrow", [2, N], BF16)        # p0 = ones, p1 = mask01
    mcolT = din("mcolT", [P, KT_N], F32)    # mask01, (kt p) -> p kt
    pjw = din("pjw", [C, C], BF16)
    w1 = din("w1", [C, C], BF16)
    w2 = din("w2", [C, C], BF16)
    scsh1 = din("scsh1", [2, C], BF16)      # p0 = sh_msa, p1 = 1+sc_msa
    scsh2 = din("scsh2", [2, C], BF16)
    osc1 = din("osc1", [1, C], BF16)        # 1+sc_msa (for the K=1 A-matmul)
    osc2 = din("osc2", [1, C], BF16)
    g1c = din("g1c", [P, CO], F32)          # g_msa, channel-major columns
    gb1c = din("gb1c", [P, CO], F32)        # g_msa*proj_b
    g2c = din("g2c", [P, CO], F32)
    gb2c = din("gb2c", [P, CO], F32)
    b1c = din("b1c", [P, CO], F32)          # mlp_b1
    qbc = din("qbc", [P, CO], F32)          # qkv_b Q rows
    kbc = din("kbc", [P, CO], F32)          # qkv_b K rows
    outT = nc.dram_tensor("outT", [C, TOK], F32, kind="ExternalOutput").ap()

    xTb_r = xTb.rearrange("(o p) n -> p o n", p=P)
    xTo_r = xTo.rearrange("(o p) n -> p o n", p=P)
    xTob_r = xTob.rearrange("(o p) n -> p o n", p=P)
    kqw_r = kqw.rearrange("(o p) m -> p o m", p=P)
    vwa_r = vwa.rearrange("(o p) m -> p o m", p=P)
    vwb_r = vwb.rearrange("(o p) m -> p o m", p=P)
    pjw_r = pjw.rearrange("(o p) m -> p o m", p=P)
    w1_r = w1.rearrange("(o p) m -> p o m", p=P)
    w2_r = w2.rearrange("(o p) m -> p o m", p=P)
    outT_r = outT.rearrange("(o p) n -> p o n", p=P)

    import contextlib

    with tile.TileContext(nc) as tc:
        loop_ctx = tc.For_i(0, loop_n, 1) if loop_n > 1 else contextlib.nullcontext()
        with loop_ctx, \
             tc.tile_pool(name="consts", bufs=1) as cst, \
             tc.tile_pool(name="res", bufs=1) as res, \
             tc.tile_pool(name="stream", bufs=2) as stm, \
             tc.tile_pool(name="rows", bufs=1) as rows:
            # ---- small constants (live whole kernel) ----
            mcolT_t = cst.tile([P, KT_N], F32, tag="mcolT")
            nc.sync.dma_start(mcolT_t[:], mcolT)
            g1c_t = cst.tile([P, CO], F32, tag="g1c")
            nc.sync.dma_start(g1c_t[:], g1c)
            gb1c_t = cst.tile([P, CO], F32, tag="gb1c")
            nc.sync.dma_start(gb1c_t[:], gb1c)
            g2c_t = cst.tile([P, CO], F32, tag="g2c")
            nc.sync.dma_start(g2c_t[:], g2c)
            gb2c_t = cst.tile([P, CO], F32, tag="gb2c")
            nc.sync.dma_start(gb2c_t[:], gb2c)
            b1c_t = cst.tile([P, CO], F32, tag="b1c")
            nc.sync.dma_start(b1c_t[:], b1c)
            qbc_t = cst.tile([P, CO], F32, tag="qbc")
            nc.sync.dma_start(qbc_t[:], qbc)
            kbc_t = cst.tile([P, CO], F32, tag="kbc")
            nc.sync.dma_start(kbc_t[:], kbc)
            onesc_t = cst.tile([P, 1], BF16, tag="onesc")
            nc.vector.memset(onesc_t[:], 1.0)
            epsc_t = cst.tile([P, 1], F32, tag="epsc")
            nc.vector.memset(epsc_t[:], EPS)
            ones64_t = cst.tile([1, 64], F32, tag="ones64")
            nc.vector.memset(ones64_t[:], 1.0)
            scsh1_t = cst.tile([2, C], BF16, tag="scsh1")
            nc.sync.dma_start(scsh1_t[:], scsh1)
            scsh2_t = cst.tile([2, C], BF16, tag="scsh2")
            nc.sync.dma_start(scsh2_t[:], scsh2)
            osc1_t = cst.tile([1, C], BF16, tag="osc1")
            nc.sync.dma_start(osc1_t[:], osc1)
            osc2_t = cst.tile([1, C], BF16, tag="osc2")
            nc.sync.dma_start(osc2_t[:], osc2)

            # ---- resident tensors ----
            XTOB = res.tile([P, CO, TOK], BF16, tag="XTOB")
            nc.sync.dma_start(XTOB[:], xTob_r)
            KT = res.tile([P, CO, N], BF16, tag="KT")
            VT = res.tile([P, KT_N, 2, 260], BF16, tag="VT")
            QT = res.tile([P, CO, TOK], BF16, tag="QT")
            OT = res.tile([P, CO, TOK], BF16, tag="OT")
            X2 = res.tile([P, CO, TOK], F32, tag="X2")

            def ln_block(psA, xb, scsh_t, osc_t, y_out):
                """LayerNorm + modulate.  xb: [P, CO, 512] bf16 AP (channel-major),
                scsh_t: [2, C] (p0=shift, p1=1+scale), y_out: [P, CO, 512] bf16 AP."""
                st = psA.tile([1, 2, 512], F32, tag="stat", bufs=1, name="st")
                for o in range(CO):
                    nc.tensor.matmul(
                        st[:, 0, :], lhsT=onesc_t[:, 0:1], rhs=xb[:, o, :],
                        start=(o == 0), stop=(o == CO - 1),
                    )
                xq = stm.tile([P, CO, 512], BF16, tag="xq", bufs=1, name="xq")
                nc.vector.tensor_mul(xq[:], xb, xb)
                for o in range(CO):
                    nc.tensor.matmul(
                        st[:, 1, :], lhsT=onesc_t[:, 0:1], rhs=xq[:, o, :],
                        start=(o == 0), stop=(o == CO - 1),
                    )
                nm = rows.tile([1, 512], F32, tag="nm", name="nm")
                nc.vector.tensor_scalar_mul(nm[:], st[0:1, 0, :], -1.0 / C)
                qq = rows.tile([1, 512], F32, tag="qq", name="qq")
                nc.vector.tensor_scalar_mul(qq[:], st[0:1, 1, :], 1.0 / C)
                v1 = rows.tile([1, 512], F32, tag="v1", name="v1")
                nc.vector.tensor_mul(v1[:], nm[:], nm[:])
                v2 = rows.tile([1, 512], F32, tag="v2", name="v2")
                nc.vector.tensor_sub(v2[:], qq[:], v1[:])
                lv = rows.tile([1, 512], F32, tag="lv", name="lv")
                nc.scalar.activation(lv[:], v2[:], AF.Ln, bias=epsc_t[0:1, :], scale=1.0)
                rs = rows.tile([1, 512], BF16, tag="rs", name="rs")
                nc.scalar.activation(rs[:], lv[:], AF.Exp, bias=0.0, scale=-0.5)
                nmr = rows.tile([1, 512], BF16, tag="nmr", name="nmr")
                nc.vector.tensor_mul(nmr[:], nm[:], rs[:])
                sr = rows.tile([2, 512], BF16, tag="sr", name="sr")
                nc.vector.memset(sr[0:1, :], 1.0)
                nc.sync.dma_start(sr[1:2, :], nmr[:])
                for o in range(CO):
                    ab = psA.tile([P, 2, 512], F32, tag="ab", bufs=2, name="ab")
                    nc.tensor.matmul(
                        ab[:, 0, :], lhsT=osc_t[0:1, o * P:(o + 1) * P], rhs=rs[:],
                        start=True, stop=True,
                    )
                    nc.tensor.matmul(
                        ab[:, 1, :], lhsT=scsh_t[0:2, o * P:(o + 1) * P], rhs=sr[:],
                        start=True, stop=True,
                    )
                    t1 = stm.tile([P, 512], BF16, tag="lt1", name="t1")
                    nc.vector.tensor_mul(t1[:], xb[:, o, :], ab[:, 0, :])
                    nc.vector.tensor_add(y_out[:, o, :], t1[:], ab[:, 1, :])

            # ================= phase 1: LN1 + K/V over full batch, Q over own =====
            with (
                tc.tile_pool(name="wA", bufs=1) as wA,
                tc.tile_pool(name="psA", bufs=1, space="PSUM") as psA,
            ):
                kqw_t = wA.tile([P, CO, 2 * C], BF16, tag="kqw")
                nc.sync.dma_start(kqw_t[:], kqw_r)
                vw_t = wA.tile([P, CO, 2, 260], BF16, tag="vw")
                nc.sync.dma_start(vw_t[:, :, 0, :], vwa_r)
                nc.sync.dma_start(vw_t[:, :, 1, :], vwb_r)
                vbr_t = wA.tile([2, 2, 260], BF16, tag="vbr")
                nc.sync.dma_start(vbr_t[:, 0, :], vbra)
                nc.sync.dma_start(vbr_t[:, 1, :], vbrb)
                mrow_t = wA.tile([2, N], BF16, tag="mrow")
                nc.sync.dma_start(mrow_t[:], mrow)

                for nt in range(NT):
                    xb = stm.tile([P, CO, 512], BF16, tag="xb", name="xb")
                    nc.sync.dma_start(xb[:], xTb_r[:, :, nt * 512:(nt + 1) * 512])
                    y = stm.tile([P, CO, 512], BF16, tag="y", name="y")
                    ln_block(psA, xb[:], scsh1_t, osc1_t, y[:])
                    # K^T columns for this n-tile
                    for r in range(CO):
                        pk = psA.tile([P, 512], F32, tag="kv", bufs=2, name="pk")
                        for o in range(CO):
                            nc.tensor.matmul(
                                pk[:],
                                lhsT=kqw_t[:, o, C + P * r: C + P * (r + 1)],
                                rhs=y[:, o, :],
                                start=(o == 0), stop=(o == CO - 1),
                            )
                        nc.vector.tensor_scalar_add(
                            KT[:, r, nt * 512:(nt + 1) * 512], pk[:], kbc_t[:, r:r + 1]
                        )
                    # V rows (token-major) for this n-tile, both halves
                    for j in range(4):
                        kt = nt * 4 + j
                        for half in range(2):
                            pv = psA.tile([P, 260], F32, tag="kv", bufs=2, name="pv")
                            for o in range(CO):
                                nc.tensor.matmul(
                                    pv[:],
                                    lhsT=y[:, o, j * P:(j + 1) * P],
                                    rhs=vw_t[:, o, half, :],
                                    start=(o == 0), stop=False,
                                )
                            nc.tensor.matmul(
                                pv[:],
                                lhsT=mrow_t[0:2, nt * 512 + j * P: nt * 512 + (j + 1) * P],
                                rhs=vbr_t[:, half, :],
                                start=False, stop=True,
                            )
                            nc.vector.tensor_scalar_mul(
                                VT[:, kt, half, :], pv[:], mcolT_t[:, kt:kt + 1]
                            )
                # Q^T from own tokens
                for nt2 in range(NT2):
                    yq = stm.tile([P, CO, 512], BF16, tag="y", name="yq")
                    ln_block(psA, XTOB[:, :, nt2 * 512:(nt2 + 1) * 512], scsh1_t, osc1_t, yq[:])
                    for r in range(CO):
                        pq = psA.tile([P, 512], F32, tag="kv", bufs=2, name="pq")
                        for o in range(CO):
                            nc.tensor.matmul(
                                pq[:],
                                lhsT=kqw_t[:, o, P * r: P * (r + 1)],
                                rhs=yq[:, o, :],
                                start=(o == 0), stop=(o == CO - 1),
                            )
                        nc.vector.tensor_scalar_add(
                            QT[:, r, nt2 * 512:(nt2 + 1) * 512], pq[:], qbc_t[:, r:r + 1]
                        )

            # ================= phase 2: attention =================
            with (
                tc.tile_pool(name="psS", bufs=2, space="PSUM") as psS,
                tc.tile_pool(name="psU", bufs=2, space="PSUM") as psU,
            ):
                for qt in range(NT2):
                    for r in range(CO):
                        half = r // 2
                        i0, i1 = (2 * r) % 4, (2 * r + 1) % 4
                        U0 = psU.tile([65, 512], F32, tag="u", name="U0")
                        U1 = psU.tile([65, 512], F32, tag="u", name="U1")
                        Us = (U0, U1)
                        vidx = (i0, i1)
                        cur = None
                        cur_e = None
                        pend = []

                        def flush():
                            nonlocal cur, cur_e, pend
                            if not pend:
                                return
                            np_ = len(pend)
                            nc.scalar.activation(
                                cur_e[:, :np_, :], cur[:, :np_, :], AF.Exp,
                                bias=0.0, scale=SCALE,
                            )
                            for (slot, uidx, kt) in pend:
                                nc.tensor.matmul(
                                    Us[uidx][:, :],
                                    lhsT=VT[:, kt, half, 65 * vidx[uidx]: 65 * vidx[uidx] + 65],
                                    rhs=cur_e[:, slot, :],
                                    start=(kt == 0), stop=(kt == KT_N - 1),
                                )
                            cur = None
                            cur_e = None
                            pend = []

                        for kt in range(KT_N):
                            for (uidx, hh) in ((0, 0), (1, 1)):
                                if cur is None:
                                    cur = psS.tile([P, EG, 512], F32, tag="s", name="scur")
                                    cur_e = stm.tile(
                                        [P, EG, 512], BF16, tag="e", bufs=3, name="ecur"
                                    )
                                slot = len(pend)
                                nc.tensor.matmul(
                                    cur[:, slot, :],
                                    lhsT=KT[64 * hh:64 * (hh + 1), r, kt * P:(kt + 1) * P],
                                    rhs=QT[64 * hh:64 * (hh + 1), r, qt * 512:(qt + 1) * 512],
                                    start=True, stop=True,
                                )
                                pend.append((slot, uidx, kt))
                                if len(pend) == EG:
                                    flush()
                        flush()
                        # divide by Z (row 64) and write o^T
                        for uidx, hh in ((0, 0), (1, 1)):
                            zi = rows.tile([1, 512], F32, tag="zi", name="zi")
                            nc.vector.reciprocal(zi[:], Us[uidx][64:65, :])
                            zbp = psS.tile([P, EG, 512], F32, tag="s", name="zbp")
                            nc.tensor.matmul(
                                zbp[0:64, 0, :], lhsT=ones64_t[:], rhs=zi[:],
                                start=True, stop=True,
                            )
                            zsb = stm.tile([64, 512], F32, tag="zsb", name="zsb")
                            nc.vector.tensor_copy(zsb[:], zbp[0:64, 0, :])
                            nc.vector.tensor_mul(
                                OT[64 * hh:64 * (hh + 1), r, qt * 512:(qt + 1) * 512],
                                Us[uidx][0:64, :], zsb[:],
                            )

            # ================= phases 3-5: proj+residual, LN2, MLP =================
            with (
                tc.tile_pool(name="wB", bufs=1) as wB,
                tc.tile_pool(name="psB", bufs=2, space="PSUM") as psB,
            ):
                pjw_t = wB.tile([P, CO, C], BF16, tag="pjw")
                nc.sync.dma_start(pjw_t[:], pjw_r)
                w1_t = wB.tile([P, CO, C], BF16, tag="w1")
                nc.sync.dma_start(w1_t[:], w1_r)
                w2_t = wB.tile([P, CO, C], BF16, tag="w2")
                nc.sync.dma_start(w2_t[:], w2_r)

                for qt in range(NT2):
                    for c2 in range(CO):
                        pp = psB.tile([P, 512], F32, tag="kv", name="pp")
                        for o in range(CO):
                            nc.tensor.matmul(
                                pp[:],
                                lhsT=pjw_t[:, o, P * c2: P * (c2 + 1)],
                                rhs=OT[:, o, qt * 512:(qt + 1) * 512],
                                start=(o == 0), stop=(o == CO - 1),
                            )
                        tp = stm.tile([P, 512], F32, tag="tp", name="tp")
                        nc.vector.tensor_scalar(
                            tp[:], pp[:], g1c_t[:, c2:c2 + 1], gb1c_t[:, c2:c2 + 1],
                            ALU.mult, ALU.add,
                        )
                        xr = stm.tile([P, 512], F32, tag="xr", name="xr")
                        nc.sync.dma_start(
                            xr[:], xTo_r[:, c2, qt * 512:(qt + 1) * 512]
                        )
                        nc.vector.tensor_add(
                            X2[:, c2, qt * 512:(qt + 1) * 512], tp[:], xr[:]
                        )

                X2B = res.tile([P, CO, TOK], BF16, tag="XTOB", name="X2B")
                nc.vector.tensor_copy(X2B[:], X2[:])
                for nt2 in range(NT2):
                    y2 = stm.tile([P, CO, 512], BF16, tag="y", name="y2")
                    ln_block(psB, X2B[:, :, nt2 * 512:(nt2 + 1) * 512], scsh2_t, osc2_t, y2[:])
                    hg = res.tile([P, CO, 512], BF16, tag="QT", name="hg")
                    for c2 in range(CO):
                        p1 = psB.tile([P, 512], F32, tag="kv", name="p1")
                        for o in range(CO):
                            nc.tensor.matmul(
                                p1[:],
                                lhsT=w1_t[:, o, P * c2: P * (c2 + 1)],
                                rhs=y2[:, o, :],
                                start=(o == 0), stop=(o == CO - 1),
                            )
                        nc.scalar.activation(
                            hg[:, c2, :], p1[:], AF.Gelu,
                            bias=b1c_t[:, c2:c2 + 1], scale=1.0,
                        )
                    for c2 in range(CO):
                        p2 = psB.tile([P, 512], F32, tag="kv", name="p2")
                        for o in range(CO):
                            nc.tensor.matmul(
                                p2[:],
                                lhsT=w2_t[:, o, P * c2: P * (c2 + 1)],
                                rhs=hg[:, o, :],
                                start=(o == 0), stop=(o == CO - 1),
                            )
                        t2 = stm.tile([P, 512], F32, tag="tp", name="t2")
                        nc.vector.tensor_scalar(
                            t2[:], p2[:], g2c_t[:, c2:c2 + 1], gb2c_t[:, c2:c2 + 1],
                            ALU.mult, ALU.add,
                        )
                        ot = stm.tile([P, 512], F32, tag="ot", name="ot")
                        nc.vector.tensor_add(
                            ot[:], t2[:], X2[:, c2, nt2 * 512:(nt2 + 1) * 512]
                        )
                        nc.sync.dma_start(
                            outT_r[:, c2, nt2 * 512:(nt2 + 1) * 512], ot[:]
                        )

    nc.compile()
    return nc


def _col(v):
    """[C] -> [P, CO] channel-major columns (c = o*P + p)."""
    return np.ascontiguousarray(np.asarray(v, np.float32).reshape(CO, P).T)


def _prep_in_maps(x, cond, mask, qkv_w, qkv_b, proj_w, proj_b, ada_w, ada_b,
                  mlp_w1, mlp_b1, mlp_w2, mlp_b2):
    f32 = np.float32
    x = np.asarray(x, f32)
    cond = np.asarray(cond, f32).reshape(B, C)
    mask = np.asarray(mask)
    qkv_w = np.asarray(qkv_w, f32)
    qkv_b = np.asarray(qkv_b, f32)
    proj_w = np.asarray(proj_w, f32)
    proj_b = np.asarray(proj_b, f32)
    ada_w = np.asarray(ada_w, f32)
    ada_b = np.asarray(ada_b, f32)
    mlp_w1 = np.asarray(mlp_w1, f32)
    mlp_b1 = np.asarray(mlp_b1, f32)
    mlp_w2 = np.asarray(mlp_w2, f32)
    mlp_b2 = np.asarray(mlp_b2, f32)

    # adaLN on host (tiny): silu(cond) @ ada_w + ada_b
    silu = cond * (1.0 / (1.0 + np.exp(-cond)))
    ada = (silu @ ada_w + ada_b).astype(f32)          # [B, 6C]
    sh1, sc1, g1, sh2, sc2, g2 = np.split(ada, 6, axis=1)

    xT = np.ascontiguousarray(x.transpose(0, 2, 1))   # [B, C, N]
    xTb = xT.astype(BF)
    m01 = (mask == 1).astype(f32)                     # [B, N]

    # V weights rearranged 65-interleaved with a zero "mask" column per head
    vw = qkv_w[:, 2 * C:3 * C]                        # [C, 512]
    vwh = np.zeros((2, C, 260), f32)
    vbr = np.zeros((2, 2, 260), f32)
    for half in range(2):
        for hh in range(4):
            h = 4 * half + hh
            vwh[half, :, 65 * hh:65 * hh + 64] = vw[:, 64 * h:64 * h + 64]
            vbr[half, 0, 65 * hh:65 * hh + 64] = qkv_b[2 * C + 64 * h: 2 * C + 64 * h + 64]
            vbr[half, 1, 65 * hh + 64] = 1.0

    shared = {
        "kqw": np.ascontiguousarray(qkv_w[:, :2 * C]).astype(BF),
        "vwa": np.ascontiguousarray(vwh[0]).astype(BF),
        "vwb": np.ascontiguousarray(vwh[1]).astype(BF),
        "vbra": np.ascontiguousarray(vbr[0]).astype(BF),
        "vbrb": np.ascontiguousarray(vbr[1]).astype(BF),
        "pjw": proj_w.astype(BF),
        "w1": mlp_w1.astype(BF),
        "w2": mlp_w2.astype(BF),
        "b1c": _col(mlp_b1),
        "qbc": _col(qkv_b[0:C]),
        "kbc": _col(qkv_b[C:2 * C]),
    }

    per_batch = []
    for b in range(B):
        pb = {
            "xTb": xTb[b],
            "mrow": np.ascontiguousarray(
                np.stack([np.ones(N, f32), m01[b]])).astype(BF),
            "mcolT": np.ascontiguousarray(m01[b].reshape(KT_N, P).T),
            "scsh1": np.ascontiguousarray(
                np.stack([sh1[b], 1.0 + sc1[b]])).astype(BF),
            "scsh2": np.ascontiguousarray(
                np.stack([sh2[b], 1.0 + sc2[b]])).astype(BF),
            "osc1": np.ascontiguousarray(1.0 + sc1[b]).reshape(1, C).astype(BF),
            "osc2": np.ascontiguousarray(1.0 + sc2[b]).reshape(1, C).astype(BF),
            "g1c": _col(g1[b]),
            "gb1c": _col(g1[b] * proj_b),
            "g2c": _col(g2[b]),
            "gb2c": _col(g2[b] * mlp_b2),
        }
        per_batch.append(pb)

    in_maps = []
    for core in range(8):
        b, s = core // 4, core % 4
        m = dict(shared)
        m.update(per_batch[b])
        xo = np.ascontiguousarray(xT[b][:, s * TOK:(s + 1) * TOK])
        m["xTo"] = xo
        m["xTob"] = xo.astype(BF)
        in_maps.append(m)
    return in_maps


def kernel(**inputs):
    global LAST_EXEC_NS
    if "nc" not in _CACHE:
        _CACHE["nc"] = _build()
    nc = _CACHE["nc"]
    in_maps = _prep_in_maps(**inputs)
    res = bass_utils.run_bass_kernel_spmd(nc, in_maps, core_ids=list(range(8)))
    LAST_EXEC_NS = res.exec_time_ns
    out = np.empty((B, N, C), np.float32)
    for core in range(8):
        b, s = core // 4, core % 4
        out[b, s * TOK:(s + 1) * TOK, :] = res.results[core]["outT"].T
    return out
